# revision 45
# baseline (speedup 1.0000x reference)
"""Fused AttentionLocal kernel for 8 Trainium2 NeuronCores — width-FFT conv1.

conv1 (7x7 dilated-2) via mixed-domain convolution: rFFT-22 along width per
column parity (the dilated taps are 7 adjacent taps on each 16-wide parity
grid), height taps stay spatial. FFT-domain matmuls in fp16 with a Karatsuba
3-mult complex multiply (t1=Wre*xre, t2=Wim*xim, t3=(Wre+Wim)*(xre+xim);
hre=t1-t2, him=t3-t1-t2) — the xre+xim plane comes straight out of the
forward DFT and Wre+Wim is folded into the host-precomputed weights.

BN stats ride a single AllReduce: the gram pass runs over [h; 1] so one
collective carries G = h h^T, s = sum(h) (extra column) and q = sum(h^2)
(diagonal, pulled back with a stride-260 DRAM view).
"""

import contextlib

import numpy as np

import concourse.bass as bass
import concourse.tile as tile
from concourse import bacc, mybir
from concourse.masks import make_identity

F32 = mybir.dt.float32
F32R = mybir.dt.float32r
BF16 = mybir.dt.bfloat16
FP16 = mybir.dt.float16
AF = mybir.ActivationFunctionType
ALU = mybir.AluOpType
AX = mybir.AxisListType
EPS = 1e-5

N_CORES = 8
B_GLOBAL = 64
C = 256
HW = 1024
W2OUT = 1024
NF = 22     # circular FFT length for 16-wide parity grid, 7-tap conv
NFREQ = 12  # rfft bins 0..11
NSLOT = 32  # 12 re + 10 im + 10 (re+im); fx 0 and 11 are real-only

KH_ORDER = [3, 0, 1, 2, 4, 5, 6]  # kh=3 (dy=0, full rows) first: start=True covers bank
KH_A = [3, 0, 1, 2]               # weight chunk A (groups 0..23)
KH_B = [4, 5, 6]                  # weight chunk B (groups 24..41)


def build_body(tc, aps, n_cores, b_loc, total_batch):
    nc = tc.nc
    P_TOT = float(total_batch * HW)
    x_ap = aps["x"]
    w1f_ap = aps["w1f"]
    w2t_ap = aps["w2t"]
    out_ap = aps["out"]

    ctx = contextlib.ExitStack()
    with ctx:
        persist = ctx.enter_context(tc.tile_pool(name="persist", bufs=1))
        dram = ctx.enter_context(tc.tile_pool(name="dram", bufs=1, space="DRAM"))

        # ---------------- prologue: constants + params ----------------
        ident16 = persist.tile([128, 128], FP16, tag="ident16", name="ident16")
        make_identity(nc, ident16[:])
        identb = persist.tile([128, 128], BF16, tag="identb", name="identb")
        make_identity(nc, identb[:])
        identf = persist.tile([128, 128], F32, tag="identf", name="identf")
        make_identity(nc, identf[:])
        ones_f32 = persist.tile([128, 2], F32, tag="ones_f32", name="ones_f32")
        nc.gpsimd.memset(ones_f32[:], 1.0)
        ones_col = persist.tile([128, 1], F32R, tag="ones_col", name="ones_col")
        nc.vector.tensor_copy(ones_col[:], ones_f32[:, 0:1])

        dfwd = persist.tile([128, 256], FP16, tag="dfwd", name="dfwd")
        nc.sync.dma_start(dfwd[:], aps["dfwd"])
        dinv = persist.tile([96, 64], FP16, tag="dinv", name="dinv")

        h_tiles = {}
        for b in range(b_loc):
            for oc in range(2):
                h_tiles[(b, oc)] = persist.tile([128, HW], BF16, tag=f"h{b}_{oc}", name=f"h{b}_{oc}")

        # xT tiles (pixel-major x, also attention operand): [128 pix, 256+2ones]
        # (ones cols prefilled after phase F so the memsets don't block the
        # gpsimd x casts at startup)
        xT = {}
        for b in range(b_loc):
            for j in range(8):
                xT[(b, j)] = persist.tile([128, 258], FP16, tag=f"xT{b}_{j}", name=f"xT{b}_{j}")

        # gram moving tiles [h; 1]: 3 rotating persistent bufs w/ ones prefilled
        hT_g = [persist.tile([128, 258], BF16, tag=f"hTg{r}", name=f"hTg{r}")
                for r in range(3)]

        bn2pp = persist.tile([128, 16], F32, tag="bn2pp", name="bn2pp")

        # fused-collective buffer: per row c (259 wide): [G[c,0:256], s[c], dup, junk]
        # diag(G)[c] sits at flat c*260 -> [256,260] view col 0 gives q = sum h^2
        gflat_in = dram.tile([256 * 260], F32, tag="gflat_in", name="gflat_in")
        gflat_out = dram.tile([256 * 260], F32, tag="gflat_out", name="gflat_out")
        g_wr_in = gflat_in[0:256 * 259].rearrange("(c k) -> c k", k=259)
        g_wr_out = gflat_out[0:256 * 259].rearrange("(c k) -> c k", k=259)
        gq_out = gflat_out[:].rearrange("(c k) -> c k", k=260)

        # hhat[oc]: [co128, (img8, rows32, g=fx*2+ri 24, par2)] fp16 — image-
        # major so the inverse transpose reads contiguous 96-col row-pair
        # chunks; the pointwise drain writes 2-elem runs (innermost par).
        with tc.tile_pool(name="hhp", bufs=1) as hhp:
            hhat = [hhp.tile([128, NFREQ * 2 * 2 * b_loc * 32], FP16,
                             tag=f"hh{oc}", name=f"hh{oc}") for oc in range(2)]
            hv = [hhat[oc][:].rearrange("p (i r g q) -> p i r g q",
                                        i=b_loc, r=32, g=2 * NFREQ, q=2)
                  for oc in range(2)]

            with tc.tile_pool(name="xhp", bufs=1) as xhp:
                # xhat[cc]: [cin, (slot32, par2, img8, rows32)] fp16
                XFREE = NSLOT * 2 * b_loc * 32
                xhat = [xhp.tile([128, XFREE], FP16, tag=f"xhat{cc}", name=f"xhat{cc}")
                        for cc in range(2)]
                xv = [xhat[cc][:].rearrange("p (s i r q) -> p s i r q",
                                            s=NSLOT, i=b_loc, r=32, q=2)
                      for cc in range(2)]
                xvm = xv

                # ------------ phase F: x transposes + forward width-DFT --------
                # pipelined by one image: img b-1's DFT matmuls are emitted
                # after img b's transposes, so the tensor queue never stalls
                # on the DVE xT copies.
                with tc.tile_pool(name="fin", bufs=2) as fin, \
                     tc.tile_pool(name="ftp", bufs=3, space="PSUM") as ftp, \
                     tc.tile_pool(name="fps", bufs=2, space="PSUM") as fps:

                    def emit_fwd(b):
                        # j-pairs share one PSUM tile; a single strided copy
                        # moves 8 consecutive row-blocks per (pair, cc)
                        for jp in range(4):
                            fo = {}
                            for cc in range(2):
                                fo[cc] = fps.tile([128, 512], F32, tag=f"fo{cc}",
                                                  name=f"fo{cc}")
                            for jj in range(2):
                                xt_ = xT[(b, jp * 2 + jj)]
                                for cc in range(2):
                                    nc.tensor.matmul(
                                        fo[cc][:, jj * 256:(jj + 1) * 256],
                                        xt_[:, cc * 128:(cc + 1) * 128], dfwd[:])
                            for cc in range(2):
                                dst = xvm[cc][:, :, b, jp * 8:(jp + 1) * 8, :] \
                                    .rearrange("p s (j r) q -> p s j r q", j=2)
                                src = fo[cc][:].rearrange("p (j s r q) -> p s j r q",
                                                          j=2, s=NSLOT, r=4)
                                if cc == 0:
                                    nc.vector.tensor_copy(dst, src)
                                else:
                                    nc.scalar.copy(dst, src)

                    for b in range(b_loc):
                        xin = {}
                        for ch in range(2):
                            for cc in range(2):
                                t = fin.tile([128, 512], F32, tag=f"xin{cc}_{ch}",
                                             name=f"xin{cc}_{ch}")
                                nc.sync.dma_start(
                                    t[:], x_ap[b, cc * 128:(cc + 1) * 128,
                                               ch * 512:(ch + 1) * 512])
                                xin[(cc, ch)] = t
                        for j in range(8):
                            xt_ = xT[(b, j)]
                            ch, jl = j // 4, j % 4
                            tp = ftp.tile([128, 256], F32, tag="tp", name="tp")
                            for cc in range(2):
                                nc.tensor.matmul(tp[:, cc * 128:(cc + 1) * 128],
                                                 xin[(cc, ch)][:, jl * 128:(jl + 1) * 128],
                                                 identf[:], is_transpose=True)
                            if j % 2 == 0:
                                nc.vector.tensor_copy(xt_[:, 0:256], tp[:])
                            else:
                                nc.scalar.copy(xt_[:, 0:256], tp[:])
                        if b > 0:
                            emit_fwd(b - 1)
                    emit_fwd(b_loc - 1)

                # ones cols for attention denominators + gram [h;1] columns
                # (emitted after phase F so gpsimd starts on the x casts)
                for b in range(b_loc):
                    for j in range(8):
                        nc.gpsimd.memset(xT[(b, j)][:, 256:258], 1.0)
                for r in range(3):
                    nc.gpsimd.memset(hT_g[r][:, 256:258], 1.0)

                # dinv not needed until phase I (off the startup critical path)
                for rr in range(2):
                    nc.sync.dma_start(dinv[rr * 48:(rr + 1) * 48, :], aps["dinv"][rr])

                # ------------ phase P: pointwise (freq-domain conv) ------------
                # Karatsuba: t1 = Wre@xre, t2 = Wim@xim, t3 = (Wre+Wim)@(xre+xim)
                # accumulated over (kh, cc); drain hre = t1-t2, him = t3-t1-t2.
                with tc.tile_pool(name="wta", bufs=2) as wta_pool, \
                     tc.tile_pool(name="wtb", bufs=1) as wtb_pool, \
                     tc.tile_pool(name="dscr", bufs=2) as dscr, \
                     tc.tile_pool(name="pps", bufs=2, space="PSUM") as pps:
                    for fx in range(NFREQ):
                        realonly = fx in (0, NFREQ - 1)
                        for co in range(2):
                            # weights: [cin, (kh wv3 cc2) x 128co] in 2 chunks
                            wtA = wta_pool.tile([128, 24 * 128], FP16, tag="wtA", name="wtA")
                            nc.sync.dma_start(
                                wtA[:].rearrange("p (g o) -> p g o", g=24),
                                w1f_ap[fx, 0:24, :, co * 128:(co + 1) * 128]
                                .rearrange("g p o -> p g o"))
                            wtB = wtb_pool.tile([128, 18 * 128], FP16, tag="wtB", name="wtB")
                            nc.sync.dma_start(
                                wtB[:].rearrange("p (g o) -> p g o", g=18),
                                w1f_ap[fx, 24:42, :, co * 128:(co + 1) * 128]
                                .rearrange("g p o -> p g o"))

                            t1 = pps.tile([128, 512], F32, tag="t1", name="t1")
                            t1v = t1[:].rearrange("p (i r q) -> p i r q", i=b_loc, r=32)
                            if not realonly:
                                t2 = pps.tile([128, 512], F32, tag="t2", name="t2")
                                t3 = pps.tile([128, 512], F32, tag="t3", name="t3")
                                t2v = t2[:].rearrange("p (i r q) -> p i r q", i=b_loc, r=32)
                                t3v = t3[:].rearrange("p (i r q) -> p i r q", i=b_loc, r=32)
                            first = True
                            for kh in KH_ORDER:
                                dy = 2 * kh - 6
                                r0 = max(0, -dy)
                                r1 = min(32, 32 - dy)
                                last = (kh == KH_ORDER[-1])
                                if kh in KH_A:
                                    wt, g0 = wtA, 0
                                else:
                                    wt, g0 = wtB, 24
                                for cc in range(2):
                                    lcc = (last and cc == 1)

                                    def wslice(wv):
                                        g = kh * 6 + wv * 2 + cc - g0
                                        return wt[:, g * 128:(g + 1) * 128]

                                    xre = xv[cc][:, fx, :, r0 + dy:r1 + dy, :]
                                    nc.tensor.matmul(t1v[:, :, r0:r1, :], wslice(0), xre,
                                                     start=first, stop=lcc,
                                                     skip_group_check=True)
                                    if not realonly:
                                        xim = xv[cc][:, 11 + fx, :, r0 + dy:r1 + dy, :]
                                        xpl = xv[cc][:, 21 + fx, :, r0 + dy:r1 + dy, :]
                                        nc.tensor.matmul(t2v[:, :, r0:r1, :], wslice(1), xim,
                                                         start=first, stop=lcc,
                                                         skip_group_check=True)
                                        nc.tensor.matmul(t3v[:, :, r0:r1, :], wslice(2), xpl,
                                                         start=first, stop=lcc,
                                                         skip_group_check=True)
                                    first = False
                            # drain into (i, r, g, q) hhat: strided dsts with
                            # 2-elem inner runs; one PSUM operand per op, so
                            # t2/t3 bounce through SBUF (scalar fast at PSUM)
                            dst0 = hv[co][:, :, :, fx * 2, :]
                            dst1 = hv[co][:, :, :, fx * 2 + 1, :]
                            if realonly:
                                # ri=1 killed by zero dinv rows; any finite filler ok
                                nc.vector.tensor_copy(dst0, t1v)
                                nc.scalar.copy(dst1, t1v)
                            else:
                                s2 = dscr.tile([128, 512], F32, tag="s2", name="s2")
                                nc.scalar.copy(s2[:], t2[:])
                                s3 = dscr.tile([128, 512], F32, tag="s3", name="s3")
                                nc.scalar.copy(s3[:], t3[:])
                                u = dscr.tile([128, 512], F32, tag="u", name="u", bufs=1)
                                nc.gpsimd.tensor_sub(u[:], s3[:], s2[:])
                                s2v = s2[:].rearrange("p (i r q) -> p i r q", i=b_loc, r=32)
                                uv = u[:].rearrange("p (i r q) -> p i r q", i=b_loc, r=32)
                                nc.vector.tensor_sub(dst0, t1v, s2v)
                                nc.vector.tensor_sub(dst1, uv, t1v)

            # ------------ phase I: inverse DFT interleaved with gram ----------
            # gram runs over [h; 1]: one AllReduce carries G, s (col 256) and
            # q = diag(G); the BN stats collectives/ops disappear.
            with tc.tile_pool(name="itp", bufs=2, space="PSUM") as itp, \
                 tc.tile_pool(name="ips", bufs=2, space="PSUM") as ips, \
                 tc.tile_pool(name="ht", bufs=4) as htpool, \
                 tc.tile_pool(name="gtp", bufs=2, space="PSUM") as gtp, \
                 tc.tile_pool(name="gscr", bufs=2) as gscr, \
                 tc.tile_pool(name="gps", bufs=1, space="PSUM") as gps_pool:
                gps = [gps_pool.tile([128, 258], F32, tag=f"gps{oc}", name=f"gps{oc}")
                       for oc in range(2)]

                def emit_inverse(b):
                    for oc in range(2):
                        for rg in range(4):  # groups of 4 rp chunks
                            tp = itp.tile([96, 512], FP16, tag="itp", name="itp")
                            for rl in range(4):
                                rp = rg * 4 + rl
                                # row-pair chunk is 96 contiguous cols
                                c0 = (b * 32 + 2 * rp) * 48
                                nc.tensor.matmul(
                                    tp[:, rl * 128:(rl + 1) * 128],
                                    hhat[oc][:, c0:c0 + 96],
                                    ident16[:], is_transpose=True)
                            hT96 = htpool.tile([96, 512], FP16, tag="hT96", name="hT96")
                            nc.vector.tensor_copy(hT96[:], tp[:])
                            io = ips.tile([128, 256], F32, tag="io", name="io")
                            for rl in range(4):
                                nc.tensor.matmul(io[:, rl * 64:(rl + 1) * 64],
                                                 hT96[:, rl * 128:(rl + 1) * 128],
                                                 dinv[:])
                            nc.scalar.copy(h_tiles[(b, oc)][:, rg * 256:(rg + 1) * 256],
                                           io[:])

                def emit_gram(b):
                    # emitted one image behind the inverse: the hT copies have
                    # a full image of slack, so gram matmuls never stall the
                    # in-order tensor queue
                    for j in range(8):
                        hT = hT_g[(b * 8 + j) % 3]
                        tpb = gtp.tile([128, 256], BF16, tag="tpb", name="tpb")
                        for oc in range(2):
                            nc.tensor.matmul(
                                tpb[:, oc * 128:(oc + 1) * 128],
                                h_tiles[(b, oc)][:, j * 128:(j + 1) * 128],
                                identb[:], is_transpose=True)
                        nc.vector.tensor_copy(hT[:, 0:256], tpb[:])
                        for oc in range(2):
                            first = (b == 0 and j == 0)
                            last = (b == b_loc - 1 and j == 7)
                            nc.tensor.matmul(
                                gps[oc][:], hT[:, oc * 128:(oc + 1) * 128], hT[:],
                                start=first, stop=last, skip_group_check=True)

                for b in range(b_loc):
                    emit_inverse(b)
                    if b > 0:
                        emit_gram(b - 1)
                emit_gram(b_loc - 1)

                for oc in range(2):
                    gsb = gscr.tile([128, 258], F32, tag=f"gsb{oc}", name=f"gsb{oc}")
                    nc.vector.tensor_copy(gsb[:], gps[oc][:])
                    nc.sync.dma_start(g_wr_in[oc * 128:(oc + 1) * 128, 0:258], gsb[:])

                nc.gpsimd.collective_compute(
                    "AllReduce", ALU.add,
                    replica_groups=[list(range(n_cores))],
                    ins=[gflat_in.opt()],
                    outs=[gflat_out.opt()])

        # late-persistent conv2 weights: loaded once xhat space is freed;
        # the DMA overlaps the collective latency
        late = ctx.enter_context(tc.tile_pool(name="late", bufs=1))
        w2t_t = []
        for cc in range(2):
            t = late.tile([128, W2OUT], F32R, tag=f"w2t{cc}", name=f"w2t{cc}")
            nc.sync.dma_start(t[:], w2t_ap[cc * 128:(cc + 1) * 128, :].bitcast(F32R))
            w2t_t.append(t)

        # ---------------- phase 3: BN constants from global stats ----------
        # column-form: everything stays in [128, k] per-partition layout so
        # there are no DRAM bounces on the post-collective critical path
        with tc.tile_pool(name="rows", bufs=1) as rows, \
             tc.tile_pool(name="prodp", bufs=2) as prodp, \
             tc.tile_pool(name="colps", bufs=1, space="PSUM") as colps, \
             tc.tile_pool(name="m1ps", bufs=2, space="PSUM") as m1ps:

            def col(name, k=2):
                return rows.tile([128, k], F32, tag=name, name=name)

            def col_load(name, src_1d, k):
                t = rows.tile([128, k], F32, tag=name, name=name)
                nc.sync.dma_start(t[:], src_1d.rearrange("(k p) -> p k", p=128))
                return t

            g1c = col_load("g1c", aps["bn1g"], 2)
            b1c = col_load("b1c", aps["bn1b"], 2)
            cb2c = col_load("cb2c", aps["cb2"], 8)
            g2pp = col_load("g2pp", aps["bn2g"], 8)
            b2pp = col_load("b2pp", aps["bn2b"], 8)

            s_col = col("s_col")
            q_col = col("q_col")
            for oc in range(2):
                nc.sync.dma_start(s_col[:, oc:oc + 1],
                                  g_wr_out[oc * 128:(oc + 1) * 128, 256:257])
                nc.sync.dma_start(q_col[:, oc:oc + 1],
                                  gq_out[oc * 128:(oc + 1) * 128, 0:1])

            meanc = col("meanc")
            nc.vector.tensor_scalar_mul(meanc[:], s_col[:], 1.0 / P_TOT)
            varc = col("varc")
            nc.vector.tensor_scalar_mul(varc[:], q_col[:], 1.0 / P_TOT)
            msqc = col("msqc")
            nc.vector.tensor_mul(msqc[:], meanc[:], meanc[:])
            nc.vector.tensor_sub(varc[:], varc[:], msqc[:])
            nc.vector.tensor_scalar_add(varc[:], varc[:], EPS)
            rec1 = col("rec1")
            nc.vector.reciprocal(rec1[:], varc[:])
            a1c = col("a1c")
            nc.scalar.activation(a1c[:], rec1[:], AF.Sqrt)
            nc.vector.tensor_mul(a1c[:], a1c[:], g1c[:])
            c1sc = col("c1sc")
            nc.vector.tensor_mul(c1sc[:], a1c[:], meanc[:])
            nc.vector.tensor_sub(c1sc[:], b1c[:], c1sc[:])

            # cst[n] = sum_c W2[n,c]*c1s[c] + conv2_b[n] (unscaled W2):
            # column outputs [128 n-chunk, 1] with lhsT = w2t chunk
            cstc = rows.tile([128, 8], F32, tag="cstc", name="cstc")
            cp_ = colps.tile([128, 8], F32, tag="cstps", name="cstps")
            for nch in range(8):
                for oc in range(2):
                    nc.tensor.matmul(
                        cp_[:, nch:nch + 1],
                        w2t_t[oc][:, nch * 128:(nch + 1) * 128].bitcast(F32),
                        c1sc[:, oc:oc + 1],
                        start=(oc == 0), stop=(oc == 1), skip_group_check=True)
            nc.vector.tensor_add(cstc[:], cp_[:], cb2c[:])

            # scale W2T in place by a1 (per input channel)
            for oc in range(2):
                nc.vector.tensor_scalar_mul(
                    w2t_t[oc][:], w2t_t[oc][:], a1c[:, oc:oc + 1])

            # r1[n] = sum_c W2'[n,c] * s[c]  (scaled W2, unscaled s)
            rp_ = colps.tile([128, 8], F32, tag="r1ps", name="r1ps")
            for nch in range(8):
                for oc in range(2):
                    nc.tensor.matmul(
                        rp_[:, nch:nch + 1],
                        w2t_t[oc][:, nch * 128:(nch + 1) * 128].bitcast(F32),
                        s_col[:, oc:oc + 1],
                        start=(oc == 0), stop=(oc == 1), skip_group_check=True)
            r1c = rows.tile([128, 8], F32, tag="r1c", name="r1c")
            nc.vector.tensor_copy(r1c[:], rp_[:])

            # M1 = G @ W2'^T ; e[n] = sum_c W2'[n,c] * M1[c,n]
            g_glob = []
            for oc in range(2):
                gg = rows.tile([128, C], F32R, tag=f"gglob{oc}", name=f"gglob{oc}")
                nc.sync.dma_start(
                    gg[:], g_wr_out[oc * 128:(oc + 1) * 128, 0:256].bitcast(F32R))
                g_glob.append(gg)
            M1 = [rows.tile([128, W2OUT], F32R, tag=f"M1_{oc}", name=f"M1_{oc}") for oc in range(2)]
            for occ in range(2):
                for nh in range(2):
                    mp = m1ps.tile([128, 512], F32, tag="m1ps", name="m1ps")
                    for dd in range(2):
                        nc.tensor.matmul(
                            mp[:], g_glob[dd][:, occ * 128:(occ + 1) * 128],
                            w2t_t[dd][:, nh * 512:(nh + 1) * 512],
                            start=(dd == 0), stop=(dd == 1), skip_group_check=True)
                    nc.vector.tensor_copy(M1[occ][:, nh * 512:(nh + 1) * 512], mp[:])
            prods = []
            for oc in range(2):
                pr = prodp.tile([128, W2OUT], F32R, tag="prod", name="prod")
                nc.vector.tensor_mul(pr[:], w2t_t[oc][:].bitcast(F32), M1[oc][:].bitcast(F32))
                prods.append(pr)
            ep_ = colps.tile([128, 8], F32, tag="eps_", name="eps_")
            for nch in range(8):
                for oc in range(2):
                    nc.tensor.matmul(
                        ep_[:, nch:nch + 1],
                        prods[oc][:, nch * 128:(nch + 1) * 128].bitcast(F32),
                        ones_f32[:, 0:1],
                        start=(oc == 0), stop=(oc == 1), skip_group_check=True)
            ec = rows.tile([128, 8], F32, tag="ec", name="ec")
            nc.vector.tensor_copy(ec[:], ep_[:])

            # BN2 constants in per-partition [128, 8] layout
            def pp(name):
                return rows.tile([128, 8], F32, tag=name, name=name)

            mkp = pp("mkp")
            nc.vector.tensor_scalar_mul(mkp[:], r1c[:], 1.0 / P_TOT)
            nc.vector.tensor_add(mkp[:], mkp[:], cstc[:])
            t1p = pp("t1p")
            nc.vector.tensor_mul(t1p[:], cstc[:], r1c[:])
            nc.vector.tensor_scalar_mul(t1p[:], t1p[:], 2.0 / P_TOT)
            t2p = pp("t2p")
            nc.vector.tensor_mul(t2p[:], cstc[:], cstc[:])
            ek2p = pp("ek2p")
            nc.vector.tensor_scalar_mul(ek2p[:], ec[:], 1.0 / P_TOT)
            nc.vector.tensor_add(ek2p[:], ek2p[:], t1p[:])
            nc.vector.tensor_add(ek2p[:], ek2p[:], t2p[:])
            nc.vector.tensor_mul(t1p[:], mkp[:], mkp[:])
            nc.vector.tensor_sub(ek2p[:], ek2p[:], t1p[:])
            nc.vector.tensor_scalar_add(ek2p[:], ek2p[:], EPS)
            nc.vector.reciprocal(t2p[:], ek2p[:])
            nc.scalar.activation(t1p[:], t2p[:], AF.Sqrt)
            nc.vector.tensor_mul(bn2pp[:, 0:8], t1p[:], g2pp[:])
            nc.vector.tensor_scalar_mul(t2p[:], r1c[:], 1.0 / P_TOT)
            nc.vector.tensor_mul(t2p[:], bn2pp[:, 0:8], t2p[:])
            nc.vector.tensor_sub(bn2pp[:, 8:16], b2pp[:], t2p[:])

        # ---------------- phase 4: conv2 + exp + attention ------------------
        with tc.tile_pool(name="w2bp", bufs=1) as w2bp, \
             tc.tile_pool(name="kexp", bufs=12) as kexp_pool, \
             tc.tile_pool(name="outp", bufs=4) as outp, \
             tc.tile_pool(name="recp", bufs=4) as recp, \
             tc.tile_pool(name="c2ps", bufs=3, space="PSUM") as c2ps, \
             tc.tile_pool(name="aps", bufs=3, space="PSUM") as aps_pool:
            # bf16 copy of the scaled conv2 weights
            w2tb = []
            for cc in range(2):
                wb = w2bp.tile([128, W2OUT], BF16, tag=f"w2tb{cc}", name=f"w2tb{cc}")
                nc.vector.tensor_copy(wb[:], w2t_t[cc][:].bitcast(F32))
                w2tb.append(wb)
            for b in range(b_loc):
                ke = []
                for j in range(8):
                    for hf in range(2):
                        cp_ = c2ps.tile([128, 512], F32, tag="c2ps", name="c2ps", bufs=3)
                        for cc in range(2):
                            nc.tensor.matmul(
                                cp_[:], w2tb[cc][:, j * 128:(j + 1) * 128],
                                h_tiles[(b, cc)][:, hf * 512:(hf + 1) * 512],
                                start=(cc == 0), stop=(cc == 1), skip_group_check=True)
                        ket = kexp_pool.tile([128, 512], FP16, tag=f"ke{hf}", name=f"ke{hf}", bufs=10)
                        nc.scalar.activation(
                            ket[:], cp_[:], AF.Exp,
                            bias=bn2pp[:, 8 + j:9 + j], scale=bn2pp[:, j:j + 1])
                        ke.append(ket)
                for pc in range(8):
                    ap_ = aps_pool.tile([128, 258], F32, tag="aps", name="aps")
                    hf, pcl = pc // 4, pc % 4
                    for j in range(8):
                        nc.tensor.matmul(
                            ap_[:], ke[j * 2 + hf][:, pcl * 128:(pcl + 1) * 128],
                            xT[(b, j)][:],
                            start=(j == 0), stop=(j == 7), skip_group_check=True)
                    rec = recp.tile([128, 1], F32, tag="rec", name="rec")
                    nc.vector.reciprocal(rec[:], ap_[:, 256:257])
                    osb = outp.tile([128, C], F32, tag="osb", name="osb")
                    nc.vector.tensor_scalar_mul(osb[:], ap_[:, 0:256], rec[:])
                    r0 = pc * 128
                    nc.sync.dma_start(out_ap[b, r0:r0 + 128, :], osb[:])


def build(n_cores=N_CORES, b_loc=B_GLOBAL // N_CORES, total_batch=B_GLOBAL):
    nc = bacc.Bacc("TRN2", target_bir_lowering=False, debug=False, num_devices=n_cores)
    aps = {
        "x": nc.dram_tensor("x", [b_loc, C, HW], F32, kind="ExternalInput").ap(),
        # w1f: [fx, (kh7 wv3 cc2)=42, cin128, co256] fp16; wv = (re, im, re+im)
        "w1f": nc.dram_tensor("w1f", [NFREQ, 42, 128, C], FP16, kind="ExternalInput").ap(),
        "dfwd": nc.dram_tensor("dfwd", [128, 256], FP16, kind="ExternalInput").ap(),
        "dinv": nc.dram_tensor("dinv", [2, 48, 64], FP16, kind="ExternalInput").ap(),
        "w2t": nc.dram_tensor("w2t", [C, W2OUT], F32, kind="ExternalInput").ap(),
        "bn1g": nc.dram_tensor("bn1g", [C], F32, kind="ExternalInput").ap(),
        "bn1b": nc.dram_tensor("bn1b", [C], F32, kind="ExternalInput").ap(),
        "bn2g": nc.dram_tensor("bn2g", [W2OUT], F32, kind="ExternalInput").ap(),
        "bn2b": nc.dram_tensor("bn2b", [W2OUT], F32, kind="ExternalInput").ap(),
        "cb2": nc.dram_tensor("cb2", [W2OUT], F32, kind="ExternalInput").ap(),
        "out": nc.dram_tensor("out", [b_loc, HW, C], F32, kind="ExternalOutput").ap(),
    }
    with tile.TileContext(nc) as tc:
        build_body(tc, aps, n_cores, b_loc, total_batch)
    nc.compile()
    return nc


_CACHE = {}


def _host_fft_consts():
    j = np.arange(16)
    fx = np.arange(NFREQ)
    ang = 2 * np.pi * np.outer(j + 3, fx) / NF          # [16, 12]
    Dre = np.cos(ang)
    Dim = -np.sin(ang)
    # dfwd [128 = (rl4 x 32cols), 256 = (slot32, rl4, par2)]
    # slots: re for fx 0..11 at fx; im for fx 1..10 at 11+fx; re+im at 21+fx
    dfwd = np.zeros((128, NSLOT, 4, 2), np.float32)
    for q in range(128):
        rl, c = q // 32, q % 32
        par, jj = c % 2, c // 2
        dfwd[q, 0:NFREQ, rl, par] = Dre[jj]
        dfwd[q, 12:22, rl, par] = Dim[jj, 1:11]
        dfwd[q, 22:32, rl, par] = Dre[jj, 1:11] + Dim[jj, 1:11]
    dfwd = dfwd.reshape(128, 256).astype(np.float16)
    # dinv [2 rowpair-halves, 48 = (fx12, ri2, par2), 64 = (rl2, 32cols)] —
    # row order matches the transposed (g=fx*2+ri, q=par) chunk layout
    angi = 2 * np.pi * np.outer(fx, j + 3) / NF          # [12, 16]
    sc = np.full((NFREQ, 1), 2.0)
    sc[0, 0] = 1.0
    sc[NFREQ - 1, 0] = 1.0
    Ire = np.cos(angi) * sc / NF
    Iim = -np.sin(angi) * sc / NF
    dinv = np.zeros((2, NFREQ, 2, 2, 2, 32), np.float32)
    for c in range(32):
        par, jj = c % 2, c // 2
        for rl in range(2):
            dinv[rl, :, 0, par, rl, c] = Ire[:, jj]
            dinv[rl, :, 1, par, rl, c] = Iim[:, jj]
    dinv = dinv.reshape(2, 48, 64).astype(np.float16)
    return dfwd, dinv


def _prep_in_maps(inputs, n_cores, b_loc):
    w1 = np.asarray(inputs["conv1_w"], np.float32)       # [co, cin, kh, kw]
    t = np.arange(7) - 3
    fx = np.arange(NFREQ)
    angw = 2 * np.pi * np.outer(t, fx) / NF
    Wre = np.cos(angw)                                    # e^{+i 2pi fx t/22}
    Wim = np.sin(angw)
    whre = np.einsum('oikt,tf->fkio', w1, Wre, optimize=True)
    whim = np.einsum('oikt,tf->fkio', w1, Wim, optimize=True)
    # w1f [fx, kh, wv3, cin256, co] -> [fx, 42, 128, co]; wv = (re, im, re+im)
    w1f = np.stack([whre, whim, whre + whim], axis=2)
    w1f = np.ascontiguousarray(w1f.reshape(NFREQ, 42, 128, C)).astype(np.float16)
    dfwd, dinv = _host_fft_consts()

    w2t = np.ascontiguousarray(np.asarray(inputs["conv2_w"], np.float32)[:, :, 0, 0].T)
    shared = {
        "w1f": w1f,
        "dfwd": dfwd,
        "dinv": dinv,
        "w2t": w2t,
        "bn1g": np.asarray(inputs["bn1_g"], np.float32),
        "bn1b": np.asarray(inputs["bn1_b"], np.float32),
        "bn2g": np.asarray(inputs["bn2_g"], np.float32),
        "bn2b": np.asarray(inputs["bn2_b"], np.float32),
        "cb2": np.asarray(inputs["conv2_b"], np.float32),
    }
    x = np.asarray(inputs["x"], np.float32).reshape(-1, C, HW)
    in_maps = []
    for i in range(n_cores):
        m = dict(shared)
        m["x"] = np.ascontiguousarray(x[i * b_loc:(i + 1) * b_loc])
        in_maps.append(m)
    return in_maps


def kernel(**inputs):
    from concourse import bass_utils
    b_loc = B_GLOBAL // N_CORES
    if "nc" not in _CACHE:
        _CACHE["nc"] = build(N_CORES, b_loc, B_GLOBAL)
    nc = _CACHE["nc"]
    in_maps = _prep_in_maps(inputs, N_CORES, b_loc)
    res = bass_utils.run_bass_kernel_spmd(nc, in_maps, core_ids=list(range(N_CORES)))
    y = np.concatenate([res.results[i]["out"] for i in range(N_CORES)], axis=0)
    return np.ascontiguousarray(y).reshape(B_GLOBAL, C, 32, 32)
